# revision 3
# baseline (speedup 1.0000x reference)
"""CrossAttentionFusion kernel for Trainium2 (8 NeuronCores, Bass/Tile).

Computation (matches the reference nn.Module):
  image_proj = relu(BN(1x1conv(image_features, image_w)))   # (B,128,H,W)
  lidar_proj = relu(BN(1x1conv(lidar_features, lidar_w)))   # (B,128,H,W)
  per (batch, 2048-pixel chunk): q = image_proj, k = v = lidar_proj
  attn_out = softmax(q k^T / sqrt(128)) @ k
  out = w0 * image_proj + w1 * attn_out,  w = softmax(modality_weights)

Sharding: the 16 independent (batch, chunk) attention problems are
distributed 2-per-core across 8 cores; each core also computes the
projections for its own pixels.  Host gathers the 8 outputs.

Per-core kernel layout notes (v2, bf16):
  - All matmul operands are bf16 (fp32 PSUM accumulate): halves the
    LDWEIGHTS cost, halves DMA, and avoids the fp32r power throttle.
  - w0 is folded into the image BN affine, w1 into the lidar BN affine;
    the exp scale compensates with 1/(w0*w1*sqrt(C)).  The final combine
    is then res = po/denom + qT.
  - Scores are k-major: sT[kpix, q] via matmul with both operands
    channel-major.  exp() on ACT writes bf16 ET.
  - The softmax denominator accumulates S += ET_i on the (otherwise
    idle) Pool engine in f32r, then ones^T @ S broadcast-sums across
    partitions on the PE.
  - AV uses transposed-K bf16 tiles: po[c, q] += Kpix_j^T @ ET_j.
  - Output is written bf16 and cast to fp32 on the host.
"""

import math
import os
import sys
from contextlib import ExitStack

import numpy as np

sys.path.insert(0, "/opt/trn_rl_repo")

import concourse.bass as bass  # noqa: E402
import concourse.tile as tile  # noqa: E402
from concourse import bacc, mybir  # noqa: E402
from concourse.bass import ds, ts  # noqa: E402
from concourse.bass_utils import run_bass_kernel_spmd  # noqa: E402

F32 = mybir.dt.float32
F32R = mybir.dt.float32r
BF16 = mybir.dt.bfloat16

B, CL, CI, CO = 2, 256, 512, 128
H = W = 128
P = H * W                    # 16384 pixels per batch
CHUNK = 2048                 # attention chunk (pixels)
NCH = P // CHUNK             # 8 chunks per batch
NCORES = 8
UPC = (B * NCH) // NCORES    # units (b,chunk) per core = 2
EPS = 1e-5
QB = 1024                    # q-block width (2 matmul halves of 512)
NQB = CHUNK // QB            # 2
KSL = CHUNK // 128           # 16 k-pixel slices per chunk
NCI_IMG = CI // 128          # 4 contraction slices for image proj
NCI_LID = CL // 128          # 2 for lidar proj

_PROGRAM = None              # compiled Bass program, built once per process
LAST_RESULTS = None          # BassKernelResults of the last kernel() call


def _build_program():
    nc = bacc.Bacc("TRN2", target_bir_lowering=False, debug=False,
                   num_devices=NCORES)

    # Per-core DRAM inputs (pre-sharded on host, bf16).
    ximg = nc.dram_tensor("ximg", [UPC, NCI_IMG, 128, CHUNK], BF16,
                          kind="ExternalInput").ap()
    xlid = nc.dram_tensor("xlid", [UPC, NCI_LID, 128, CHUNK], BF16,
                          kind="ExternalInput").ap()
    wimg = nc.dram_tensor("wimg", [NCI_IMG, 128, CO], BF16,
                          kind="ExternalInput").ap()
    wlid = nc.dram_tensor("wlid", [NCI_LID, 128, CO], BF16,
                          kind="ExternalInput").ap()
    img_scale = nc.dram_tensor("img_scale", [CO, 1], F32, kind="ExternalInput").ap()
    img_bias = nc.dram_tensor("img_bias", [CO, 1], F32, kind="ExternalInput").ap()
    lid_scale = nc.dram_tensor("lid_scale", [CO, 1], F32, kind="ExternalInput").ap()
    lid_bias = nc.dram_tensor("lid_bias", [CO, 1], F32, kind="ExternalInput").ap()
    escale = nc.dram_tensor("escale", [128, 1], F32, kind="ExternalInput").ap()
    ident = nc.dram_tensor("ident", [128, 128], BF16, kind="ExternalInput").ap()
    ones_m = nc.dram_tensor("ones_m", [128, 128], F32R, kind="ExternalInput").ap()
    y = nc.dram_tensor("y", [UPC, CO, CHUNK], BF16, kind="ExternalOutput").ap()

    with tile.TileContext(nc) as tc, ExitStack() as ctx:
        const = ctx.enter_context(tc.tile_pool(name="const", bufs=1))
        xi_pool = ctx.enter_context(tc.tile_pool(name="xi", bufs=6))
        xl_pool = ctx.enter_context(tc.tile_pool(name="xl", bufs=2 * NCI_LID))
        proj_pool = ctx.enter_context(tc.tile_pool(name="proj", bufs=2))
        kp_pool = ctx.enter_context(tc.tile_pool(name="kp", bufs=4))
        et_pool = ctx.enter_context(tc.tile_pool(name="et", bufs=4))
        s_pool = ctx.enter_context(tc.tile_pool(name="s", bufs=2))
        misc_pool = ctx.enter_context(tc.tile_pool(name="misc", bufs=2))
        res_pool = ctx.enter_context(tc.tile_pool(name="res", bufs=2))
        # PSUM: mm 2x[128,1024]f32 (4 banks) + av 2x[128,1024]f32 (4 banks).
        # Transposes (bf16) and the denominator matmul borrow mm slots.
        mm_psum = ctx.enter_context(tc.tile_pool(name="mmps", bufs=2, space="PSUM"))
        av_psum = ctx.enter_context(tc.tile_pool(name="avps", bufs=2, space="PSUM"))

        # constants
        wimg_t = const.tile([128, NCI_IMG * CO], BF16)
        for ci in range(NCI_IMG):
            nc.sync.dma_start(wimg_t[:, ts(ci, CO)], wimg[ci])
        wlid_t = const.tile([128, NCI_LID * CO], BF16)
        for ci in range(NCI_LID):
            nc.sync.dma_start(wlid_t[:, ts(ci, CO)], wlid[ci])
        img_s = const.tile([128, 1], F32)
        nc.sync.dma_start(img_s[:], img_scale)
        img_b = const.tile([128, 1], F32)
        nc.sync.dma_start(img_b[:], img_bias)
        lid_s = const.tile([128, 1], F32)
        nc.sync.dma_start(lid_s[:], lid_scale)
        lid_b = const.tile([128, 1], F32)
        nc.sync.dma_start(lid_b[:], lid_bias)
        esc = const.tile([128, 1], F32)
        nc.sync.dma_start(esc[:], escale)
        ident_t = const.tile([128, 128], BF16)
        nc.sync.dma_start(ident_t[:], ident)
        ones_t = const.tile([128, 128], F32R)
        nc.sync.dma_start(ones_t[:], ones_m)

        for u in range(UPC):
            # ---- load unit inputs ----
            xi = []
            for ci in range(NCI_IMG):
                t = xi_pool.tile([128, CHUNK], BF16, name=f"xi_{u}_{ci}", tag="xi")
                for hh in range(2):
                    nc.sync.dma_start(t[:, ts(hh, QB)], ximg[u, ci, :, ts(hh, QB)])
                xi.append(t)
            xl = []
            for ci in range(NCI_LID):
                t = xl_pool.tile([128, CHUNK], BF16, name=f"xl_{u}_{ci}", tag="xl")
                for hh in range(2):
                    nc.sync.dma_start(t[:, ts(hh, QB)], xlid[u, ci, :, ts(hh, QB)])
                xl.append(t)

            # ---- projections (channel-major), QB-wide PSUM, halves of 512 ----
            qT = proj_pool.tile([128, CHUNK], BF16, name=f"qT_{u}", tag="qT")
            kT = proj_pool.tile([128, CHUNK], BF16, name=f"kT_{u}", tag="kT")
            for qb in range(NQB):
                ps = mm_psum.tile([128, QB], F32, name=f"psi_{u}_{qb}", tag="mm")
                for h in range(QB // 512):
                    for ci in range(NCI_IMG):
                        nc.tensor.matmul(ps[:, ts(h, 512)], wimg_t[:, ts(ci, CO)],
                                         xi[ci][:, ds(qb * QB + h * 512, 512)],
                                         start=(ci == 0), stop=(ci == NCI_IMG - 1))
                nc.scalar.activation(qT[:, ts(qb, QB)], ps[:],
                                     mybir.ActivationFunctionType.Relu,
                                     bias=img_b[:], scale=img_s[:])
                ps2 = mm_psum.tile([128, QB], F32, name=f"psl_{u}_{qb}", tag="mm")
                for h in range(QB // 512):
                    for ci in range(NCI_LID):
                        nc.tensor.matmul(ps2[:, ts(h, 512)], wlid_t[:, ts(ci, CO)],
                                         xl[ci][:, ds(qb * QB + h * 512, 512)],
                                         start=(ci == 0), stop=(ci == NCI_LID - 1))
                nc.scalar.activation(kT[:, ts(qb, QB)], ps2[:],
                                     mybir.ActivationFunctionType.Relu,
                                     bias=lid_b[:], scale=lid_s[:])

            # ---- transpose K to pixel-major bf16 tiles (8 transposes per
            # one-bank bf16 PSUM tile borrowed from the mm pool, one wide
            # DVE copy) ----
            kpw = []
            for g in range(KSL // 8):
                pt = mm_psum.tile([128, 8 * 128], BF16, name=f"pt_{u}_{g}",
                                  tag="mm")
                for k in range(8):
                    nc.tensor.transpose(pt[:, ts(k, 128)],
                                        kT[:, ts(g * 8 + k, 128)], ident_t[:])
                kpt = kp_pool.tile([128, 8 * 128], BF16,
                                   name=f"kp_{u}_{g}", tag="kp")
                nc.vector.tensor_copy(kpt[:], pt[:])
                kpw.append(kpt)

            # ---- attention, one q-block at a time ----
            res_u = res_pool.tile([128, CHUNK], BF16, name=f"res_{u}", tag="res")
            LOOKAHEAD = 2  # AV matmuls lag scores so the in-order PE queue
            #                never stalls waiting on ACT-engine exp
            for qb in range(NQB):
                po = av_psum.tile([128, QB], F32, name=f"po_{u}_{qb}", tag="av")
                S = s_pool.tile([128, QB], F32R, name=f"S_{u}_{qb}", tag="S")
                ets = [None] * KSL
                for i in range(KSL + LOOKAHEAD):
                    if i < KSL:
                        ps = mm_psum.tile([128, QB], F32,
                                          name=f"pss_{u}_{qb}_{i}", tag="mm")
                        for h in range(QB // 512):
                            nc.tensor.matmul(ps[:, ts(h, 512)], kT[:, ts(i, 128)],
                                             qT[:, ds(qb * QB + h * 512, 512)],
                                             start=True, stop=True)
                        et = et_pool.tile([128, QB], BF16,
                                          name=f"et_{u}_{qb}_{i}", tag="et")
                        nc.scalar.activation(et[:], ps[:],
                                             mybir.ActivationFunctionType.Exp,
                                             scale=esc[:])
                        ets[i] = et
                        # softmax denominator partial sums on the Pool engine
                        if i == 0:
                            nc.gpsimd.tensor_copy(S[:], et[:])
                        else:
                            nc.gpsimd.tensor_add(S[:], S[:], et[:])
                    j = i - LOOKAHEAD
                    if j >= 0:
                        kslice = kpw[j // 8][:, ts(j % 8, 128)]
                        for h in range(QB // 512):
                            nc.tensor.matmul(po[:, ts(h, 512)], kslice,
                                             ets[j][:, ts(h, 512)],
                                             start=(j == 0), stop=(j == KSL - 1))
                pl = mm_psum.tile([128, QB], F32, name=f"pl_{u}_{qb}", tag="mm")
                for h in range(QB // 512):
                    nc.tensor.matmul(pl[:, ts(h, 512)], ones_t[:],
                                     S[:, ts(h, 512)], start=True, stop=True)
                linv = misc_pool.tile([128, QB], F32, name=f"linv_{u}_{qb}",
                                      tag="linv")
                nc.vector.reciprocal_approx_fast(linv[:], pl[:])
                tmp = misc_pool.tile([128, QB], F32, name=f"tmp_{u}_{qb}",
                                     tag="tmp")
                nc.vector.tensor_mul(tmp[:], po[:], linv[:])
                # res = attn_out + w0*image_proj  (qT has w0, po/S has w1)
                nc.gpsimd.tensor_add(res_u[:, ts(qb, QB)], tmp[:],
                                     qT[:, ts(qb, QB)])
            nc.sync.dma_start(y[u], res_u[:])

    nc.compile()
    return nc


def _to_bf16(a):
    """Round-to-nearest-even fp32 -> bf16, returned as ml_dtypes.bfloat16."""
    import ml_dtypes
    return np.asarray(a, np.float32).astype(ml_dtypes.bfloat16)


def _shard_inputs(inputs):
    """Build the 8 per-core input maps from the full input dict."""
    import ml_dtypes
    mw = np.asarray(inputs["modality_weights"], np.float64)
    e = np.exp(mw - mw.max())
    w = (e / e.sum()).astype(np.float64)
    w0, w1 = float(w[0]), float(w[1])

    def bn_fold(gamma, beta, mean, var, mul):
        g = np.asarray(gamma, np.float64)
        b = np.asarray(beta, np.float64)
        m = np.asarray(mean, np.float64)
        v = np.asarray(var, np.float64)
        scale = g / np.sqrt(v + EPS) * mul
        bias = (b - m * g / np.sqrt(v + EPS)) * mul
        return (scale.astype(np.float32).reshape(CO, 1),
                bias.astype(np.float32).reshape(CO, 1))

    i_s, i_b = bn_fold(inputs["image_gamma"], inputs["image_beta"],
                       inputs["image_mean"], inputs["image_var"], w0)
    l_s, l_b = bn_fold(inputs["lidar_gamma"], inputs["lidar_beta"],
                       inputs["lidar_mean"], inputs["lidar_var"], w1)

    # weight slices, pre-transposed for lhsT ([cin_slice, cout]), bf16
    wi = _to_bf16(np.ascontiguousarray(
        np.asarray(inputs["image_w"], np.float32).T.reshape(NCI_IMG, 128, CO)))
    wl = _to_bf16(np.ascontiguousarray(
        np.asarray(inputs["lidar_w"], np.float32).T.reshape(NCI_LID, 128, CO)))

    esc = np.full((128, 1), 1.0 / (w0 * w1 * math.sqrt(CO)), np.float32)
    ident = np.eye(128, dtype=ml_dtypes.bfloat16)
    ones_m = np.ones((128, 128), np.float32)

    # full features reshaped to (B, nchunks, C, 2048), cast once to bf16
    img = _to_bf16(np.asarray(inputs["image_features"], np.float32)
                   ).reshape(B, CI, NCH, CHUNK)
    lid = _to_bf16(np.asarray(inputs["lidar_features"], np.float32)
                   ).reshape(B, CL, NCH, CHUNK)

    in_maps = []
    for core in range(NCORES):
        ximg = np.empty((UPC, NCI_IMG, 128, CHUNK), ml_dtypes.bfloat16)
        xlid = np.empty((UPC, NCI_LID, 128, CHUNK), ml_dtypes.bfloat16)
        for ul in range(UPC):
            un = core * UPC + ul
            b, c = un // NCH, un % NCH
            ximg[ul] = img[b, :, c, :].reshape(NCI_IMG, 128, CHUNK)
            xlid[ul] = lid[b, :, c, :].reshape(NCI_LID, 128, CHUNK)
        in_maps.append({
            "ximg": ximg, "xlid": xlid, "wimg": wi, "wlid": wl,
            "img_scale": i_s, "img_bias": i_b,
            "lid_scale": l_s, "lid_bias": l_b,
            "escale": esc, "ident": ident, "ones_m": ones_m,
        })
    return in_maps


def kernel(**inputs) -> np.ndarray:
    global _PROGRAM, LAST_RESULTS
    if _PROGRAM is None:
        _PROGRAM = _build_program()
    nc = _PROGRAM

    in_maps = _shard_inputs(inputs)
    trace = os.environ.get("BASS_KERNEL_TRACE", "0") == "1"
    tmpdir = os.environ.get("BASS_KERNEL_TRACE_DIR") or None
    if tmpdir:
        os.makedirs(tmpdir, exist_ok=True)
    results = run_bass_kernel_spmd(nc, in_maps, core_ids=list(range(NCORES)),
                                   trace=trace, tmpdir=tmpdir)
    LAST_RESULTS = results

    out = np.empty((B, CO, H, W), np.float32)
    outv = out.reshape(B, CO, NCH, CHUNK)
    for core in range(NCORES):
        yc = results.results[core]["y"]
        for ul in range(UPC):
            un = core * UPC + ul
            b, c = un // NCH, un % NCH
            outv[b, :, c, :] = np.asarray(yc[ul], np.float32)
    return out


if __name__ == "__main__":
    rng = np.random.default_rng(0)
    inputs = {
        "lidar_features": rng.standard_normal((B, CL, H, W), np.float32),
        "image_features": rng.standard_normal((B, CI, H, W), np.float32),
        "lidar_w": rng.standard_normal((CO, CL), np.float32) * np.sqrt(2.0 / CO),
        "lidar_gamma": np.ones(CO, np.float32),
        "lidar_beta": np.zeros(CO, np.float32),
        "lidar_mean": rng.standard_normal(CO).astype(np.float32) * 0.1,
        "lidar_var": rng.uniform(0.5, 1.5, CO).astype(np.float32),
        "image_w": rng.standard_normal((CO, CI), np.float32) * np.sqrt(2.0 / CO),
        "image_gamma": np.ones(CO, np.float32),
        "image_beta": np.zeros(CO, np.float32),
        "image_mean": rng.standard_normal(CO).astype(np.float32) * 0.1,
        "image_var": rng.uniform(0.5, 1.5, CO).astype(np.float32),
        "modality_weights": np.ones(2, np.float32),
    }
    out = kernel(**inputs)
    print("kernel out:", out.shape, out.dtype, float(np.abs(out).mean()))


# revision 5
# speedup vs baseline: 1.2269x; 1.2269x over previous
"""CrossAttentionFusion kernel for Trainium2 (8 NeuronCores, Bass/Tile).

Computation (matches the reference nn.Module):
  image_proj = relu(BN(1x1conv(image_features, image_w)))   # (B,128,H,W)
  lidar_proj = relu(BN(1x1conv(lidar_features, lidar_w)))   # (B,128,H,W)
  per (batch, 2048-pixel chunk): q = image_proj, k = v = lidar_proj
  attn_out = softmax(q k^T / sqrt(128)) @ k
  out = w0 * image_proj + w1 * attn_out,  w = softmax(modality_weights)

Sharding: the 16 independent (batch, chunk) attention problems are
distributed 2-per-core across 8 cores; each core also computes the
projections for its own pixels.  Host gathers the 8 outputs.

Per-core kernel layout notes (v2, bf16):
  - All matmul operands are bf16 (fp32 PSUM accumulate): halves the
    LDWEIGHTS cost, halves DMA, and avoids the fp32r power throttle.
  - w0 is folded into the image BN affine, w1 into the lidar BN affine;
    the exp scale compensates with 1/(w0*w1*sqrt(C)).  The final combine
    is then res = po/denom + qT.
  - Scores are k-major: sT[kpix, q] via matmul with both operands
    channel-major.  exp() on ACT writes bf16 ET.
  - The softmax denominator accumulates S += ET_i on the (otherwise
    idle) Pool engine in f32r, then ones^T @ S broadcast-sums across
    partitions on the PE.
  - AV uses transposed-K bf16 tiles: po[c, q] += Kpix_j^T @ ET_j.
  - Output is written bf16 and cast to fp32 on the host.
"""

import math
import os
import sys
from contextlib import ExitStack

import numpy as np

sys.path.insert(0, "/opt/trn_rl_repo")

import concourse.bass as bass  # noqa: E402
import concourse.tile as tile  # noqa: E402
from concourse import bacc, mybir  # noqa: E402
from concourse.bass import ds, ts  # noqa: E402
from concourse.bass_utils import run_bass_kernel_spmd  # noqa: E402

F32 = mybir.dt.float32
F32R = mybir.dt.float32r
BF16 = mybir.dt.bfloat16

B, CL, CI, CO = 2, 256, 512, 128
H = W = 128
P = H * W                    # 16384 pixels per batch
CHUNK = 2048                 # attention chunk (pixels)
NCH = P // CHUNK             # 8 chunks per batch
NCORES = 8
UPC = (B * NCH) // NCORES    # units (b,chunk) per core = 2
EPS = 1e-5
QB = 1024                    # q-block width (2 matmul halves of 512)
NQB = CHUNK // QB            # 2
KSL = CHUNK // 128           # 16 k-pixel slices per chunk
NCI_IMG = CI // 128          # 4 contraction slices for image proj
NCI_LID = CL // 128          # 2 for lidar proj

_PROGRAM = None              # compiled Bass program, built once per process
LAST_RESULTS = None          # BassKernelResults of the last kernel() call


def _build_program():
    nc = bacc.Bacc("TRN2", target_bir_lowering=False, debug=False,
                   num_devices=NCORES)

    # Per-core DRAM inputs (pre-sharded on host, bf16).
    ximg = nc.dram_tensor("ximg", [UPC, NCI_IMG, 128, CHUNK], BF16,
                          kind="ExternalInput").ap()
    xlid = nc.dram_tensor("xlid", [UPC, NCI_LID, 128, CHUNK], BF16,
                          kind="ExternalInput").ap()
    wimg = nc.dram_tensor("wimg", [NCI_IMG, 128, CO], BF16,
                          kind="ExternalInput").ap()
    wlid = nc.dram_tensor("wlid", [NCI_LID, 128, CO], BF16,
                          kind="ExternalInput").ap()
    img_scale = nc.dram_tensor("img_scale", [CO, 1], F32, kind="ExternalInput").ap()
    img_bias = nc.dram_tensor("img_bias", [CO, 1], F32, kind="ExternalInput").ap()
    lid_scale = nc.dram_tensor("lid_scale", [CO, 1], F32, kind="ExternalInput").ap()
    lid_bias = nc.dram_tensor("lid_bias", [CO, 1], F32, kind="ExternalInput").ap()
    escale = nc.dram_tensor("escale", [128, 1], F32, kind="ExternalInput").ap()
    ident = nc.dram_tensor("ident", [128, 128], BF16, kind="ExternalInput").ap()
    ones_m = nc.dram_tensor("ones_m", [128, 128], F32R, kind="ExternalInput").ap()
    y = nc.dram_tensor("y", [UPC, CO, CHUNK], BF16, kind="ExternalOutput").ap()

    with tile.TileContext(nc) as tc, ExitStack() as ctx:
        const = ctx.enter_context(tc.tile_pool(name="const", bufs=1))
        xi_pool = ctx.enter_context(tc.tile_pool(name="xi", bufs=6))
        xl_pool = ctx.enter_context(tc.tile_pool(name="xl", bufs=2 * NCI_LID))
        proj_pool = ctx.enter_context(tc.tile_pool(name="proj", bufs=4))
        kp_pool = ctx.enter_context(tc.tile_pool(name="kp", bufs=4))
        et_pool = ctx.enter_context(tc.tile_pool(name="et", bufs=6))
        s_pool = ctx.enter_context(tc.tile_pool(name="s", bufs=4))
        misc_pool = ctx.enter_context(tc.tile_pool(name="misc", bufs=2))
        res_pool = ctx.enter_context(tc.tile_pool(name="res", bufs=2))
        # PSUM: mm 2x[128,1024]f32 (4 banks) + av 2x[128,1024]f32 (4 banks).
        # Transposes (bf16) and the denominator matmul borrow mm slots.
        mm_psum = ctx.enter_context(tc.tile_pool(name="mmps", bufs=2, space="PSUM"))
        av_psum = ctx.enter_context(tc.tile_pool(name="avps", bufs=2, space="PSUM"))

        # constants
        wimg_t = const.tile([128, NCI_IMG * CO], BF16)
        for ci in range(NCI_IMG):
            nc.sync.dma_start(wimg_t[:, ts(ci, CO)], wimg[ci])
        wlid_t = const.tile([128, NCI_LID * CO], BF16)
        for ci in range(NCI_LID):
            nc.sync.dma_start(wlid_t[:, ts(ci, CO)], wlid[ci])
        img_s = const.tile([128, 1], F32)
        nc.sync.dma_start(img_s[:], img_scale)
        img_b = const.tile([128, 1], F32)
        nc.sync.dma_start(img_b[:], img_bias)
        lid_s = const.tile([128, 1], F32)
        nc.sync.dma_start(lid_s[:], lid_scale)
        lid_b = const.tile([128, 1], F32)
        nc.sync.dma_start(lid_b[:], lid_bias)
        esc = const.tile([128, 1], F32)
        nc.sync.dma_start(esc[:], escale)
        ident_t = const.tile([128, 128], BF16)
        nc.sync.dma_start(ident_t[:], ident)
        ones_t = const.tile([128, 128], F32R)
        nc.sync.dma_start(ones_t[:], ones_m)

        for u in range(UPC):
            # ---- load unit inputs ----
            xi = []
            for ci in range(NCI_IMG):
                t = xi_pool.tile([128, CHUNK], BF16, name=f"xi_{u}_{ci}", tag="xi")
                for hh in range(2):
                    nc.sync.dma_start(t[:, ts(hh, QB)], ximg[u, ci, :, ts(hh, QB)])
                xi.append(t)
            xl = []
            for ci in range(NCI_LID):
                t = xl_pool.tile([128, CHUNK], BF16, name=f"xl_{u}_{ci}", tag="xl")
                for hh in range(2):
                    nc.sync.dma_start(t[:, ts(hh, QB)], xlid[u, ci, :, ts(hh, QB)])
                xl.append(t)

            # ---- projections (channel-major), QB-wide PSUM, halves of 512 ----
            qT = proj_pool.tile([128, CHUNK], BF16, name=f"qT_{u}", tag="qT")
            kT = proj_pool.tile([128, CHUNK], BF16, name=f"kT_{u}", tag="kT")
            for qb in range(NQB):
                ps = mm_psum.tile([128, QB], F32, name=f"psi_{u}_{qb}", tag="mm")
                for h in range(QB // 512):
                    for ci in range(NCI_IMG):
                        nc.tensor.matmul(ps[:, ts(h, 512)], wimg_t[:, ts(ci, CO)],
                                         xi[ci][:, ds(qb * QB + h * 512, 512)],
                                         start=(ci == 0), stop=(ci == NCI_IMG - 1))
                nc.scalar.activation(qT[:, ts(qb, QB)], ps[:],
                                     mybir.ActivationFunctionType.Relu,
                                     bias=img_b[:], scale=img_s[:])
                ps2 = mm_psum.tile([128, QB], F32, name=f"psl_{u}_{qb}", tag="mm")
                for h in range(QB // 512):
                    for ci in range(NCI_LID):
                        nc.tensor.matmul(ps2[:, ts(h, 512)], wlid_t[:, ts(ci, CO)],
                                         xl[ci][:, ds(qb * QB + h * 512, 512)],
                                         start=(ci == 0), stop=(ci == NCI_LID - 1))
                nc.scalar.activation(kT[:, ts(qb, QB)], ps2[:],
                                     mybir.ActivationFunctionType.Relu,
                                     bias=lid_b[:], scale=lid_s[:])

            # ---- transpose K to pixel-major bf16 tiles (8 transposes per
            # one-bank bf16 PSUM tile borrowed from the mm pool, one wide
            # DVE copy) ----
            kpw = []
            for g in range(KSL // 8):
                pt = mm_psum.tile([128, 8 * 128], BF16, name=f"pt_{u}_{g}",
                                  tag="mm")
                for k in range(8):
                    nc.tensor.transpose(pt[:, ts(k, 128)],
                                        kT[:, ts(g * 8 + k, 128)], ident_t[:])
                kpt = kp_pool.tile([128, 8 * 128], BF16,
                                   name=f"kp_{u}_{g}", tag="kp")
                nc.vector.tensor_copy(kpt[:], pt[:])
                kpw.append(kpt)

            # ---- attention, one q-block at a time ----
            res_u = res_pool.tile([128, CHUNK], BF16, name=f"res_{u}", tag="res")
            LOOKAHEAD = 3  # AV matmuls lag scores so the in-order PE queue
            #                never stalls waiting on ACT-engine exp
            NDVE = 10      # denominator slices on DVE; the rest on Pool
            #                (Pool's tensor ops run ~2x slower than DVE's)
            for qb in range(NQB):
                po = av_psum.tile([128, QB], F32, name=f"po_{u}_{qb}", tag="av")
                SA = s_pool.tile([128, QB], F32R, name=f"SA_{u}_{qb}", tag="S")
                SB = s_pool.tile([128, QB], F32R, name=f"SB_{u}_{qb}", tag="S")
                ets = [None] * KSL
                for i in range(KSL + LOOKAHEAD):
                    if i < KSL:
                        ps = mm_psum.tile([128, QB], F32,
                                          name=f"pss_{u}_{qb}_{i}", tag="mm")
                        for h in range(QB // 512):
                            nc.tensor.matmul(ps[:, ts(h, 512)], kT[:, ts(i, 128)],
                                             qT[:, ds(qb * QB + h * 512, 512)],
                                             start=True, stop=True)
                        et = et_pool.tile([128, QB], BF16,
                                          name=f"et_{u}_{qb}_{i}", tag="et")
                        nc.scalar.activation(et[:], ps[:],
                                             mybir.ActivationFunctionType.Exp,
                                             scale=esc[:])
                        ets[i] = et
                        # softmax denominator partial sums, split across the
                        # DVE and Pool engines to keep both off the critical
                        # path of the slice pipeline
                        if i < NDVE:
                            if i == 0:
                                nc.vector.tensor_copy(SA[:], et[:])
                            else:
                                nc.vector.tensor_add(SA[:], SA[:], et[:])
                        else:
                            if i == NDVE:
                                nc.gpsimd.tensor_copy(SB[:], et[:])
                            else:
                                nc.gpsimd.tensor_add(SB[:], SB[:], et[:])
                    j = i - LOOKAHEAD
                    if j >= 0:
                        kslice = kpw[j // 8][:, ts(j % 8, 128)]
                        for h in range(QB // 512):
                            nc.tensor.matmul(po[:, ts(h, 512)], kslice,
                                             ets[j][:, ts(h, 512)],
                                             start=(j == 0), stop=(j == KSL - 1))
                pl = mm_psum.tile([128, QB], F32, name=f"pl_{u}_{qb}", tag="mm")
                for h in range(QB // 512):
                    nc.tensor.matmul(pl[:, ts(h, 512)], ones_t[:],
                                     SA[:, ts(h, 512)], start=True, stop=False)
                    nc.tensor.matmul(pl[:, ts(h, 512)], ones_t[:],
                                     SB[:, ts(h, 512)], start=False, stop=True)
                linv = misc_pool.tile([128, QB], F32, name=f"linv_{u}_{qb}",
                                      tag="linv")
                nc.vector.reciprocal_approx_fast(linv[:], pl[:])
                tmp = misc_pool.tile([128, QB], F32, name=f"tmp_{u}_{qb}",
                                     tag="tmp")
                nc.vector.tensor_mul(tmp[:], po[:], linv[:])
                # res = attn_out + w0*image_proj  (qT has w0, po/S has w1)
                nc.gpsimd.tensor_add(res_u[:, ts(qb, QB)], tmp[:],
                                     qT[:, ts(qb, QB)])
            nc.sync.dma_start(y[u], res_u[:])

    nc.compile()
    return nc


def _to_bf16(a):
    """Round-to-nearest-even fp32 -> bf16, returned as ml_dtypes.bfloat16."""
    import ml_dtypes
    return np.asarray(a, np.float32).astype(ml_dtypes.bfloat16)


def _shard_inputs(inputs):
    """Build the 8 per-core input maps from the full input dict."""
    import ml_dtypes
    mw = np.asarray(inputs["modality_weights"], np.float64)
    e = np.exp(mw - mw.max())
    w = (e / e.sum()).astype(np.float64)
    w0, w1 = float(w[0]), float(w[1])

    def bn_fold(gamma, beta, mean, var, mul):
        g = np.asarray(gamma, np.float64)
        b = np.asarray(beta, np.float64)
        m = np.asarray(mean, np.float64)
        v = np.asarray(var, np.float64)
        scale = g / np.sqrt(v + EPS) * mul
        bias = (b - m * g / np.sqrt(v + EPS)) * mul
        return (scale.astype(np.float32).reshape(CO, 1),
                bias.astype(np.float32).reshape(CO, 1))

    i_s, i_b = bn_fold(inputs["image_gamma"], inputs["image_beta"],
                       inputs["image_mean"], inputs["image_var"], w0)
    l_s, l_b = bn_fold(inputs["lidar_gamma"], inputs["lidar_beta"],
                       inputs["lidar_mean"], inputs["lidar_var"], w1)

    # weight slices, pre-transposed for lhsT ([cin_slice, cout]), bf16
    wi = _to_bf16(np.ascontiguousarray(
        np.asarray(inputs["image_w"], np.float32).T.reshape(NCI_IMG, 128, CO)))
    wl = _to_bf16(np.ascontiguousarray(
        np.asarray(inputs["lidar_w"], np.float32).T.reshape(NCI_LID, 128, CO)))

    esc = np.full((128, 1), 1.0 / (w0 * w1 * math.sqrt(CO)), np.float32)
    ident = np.eye(128, dtype=ml_dtypes.bfloat16)
    ones_m = np.ones((128, 128), np.float32)

    # full features reshaped to (B, nchunks, C, 2048), cast once to bf16
    img = _to_bf16(np.asarray(inputs["image_features"], np.float32)
                   ).reshape(B, CI, NCH, CHUNK)
    lid = _to_bf16(np.asarray(inputs["lidar_features"], np.float32)
                   ).reshape(B, CL, NCH, CHUNK)

    in_maps = []
    for core in range(NCORES):
        ximg = np.empty((UPC, NCI_IMG, 128, CHUNK), ml_dtypes.bfloat16)
        xlid = np.empty((UPC, NCI_LID, 128, CHUNK), ml_dtypes.bfloat16)
        for ul in range(UPC):
            un = core * UPC + ul
            b, c = un // NCH, un % NCH
            ximg[ul] = img[b, :, c, :].reshape(NCI_IMG, 128, CHUNK)
            xlid[ul] = lid[b, :, c, :].reshape(NCI_LID, 128, CHUNK)
        in_maps.append({
            "ximg": ximg, "xlid": xlid, "wimg": wi, "wlid": wl,
            "img_scale": i_s, "img_bias": i_b,
            "lid_scale": l_s, "lid_bias": l_b,
            "escale": esc, "ident": ident, "ones_m": ones_m,
        })
    return in_maps


def kernel(**inputs) -> np.ndarray:
    global _PROGRAM, LAST_RESULTS
    if _PROGRAM is None:
        _PROGRAM = _build_program()
    nc = _PROGRAM

    in_maps = _shard_inputs(inputs)
    trace = os.environ.get("BASS_KERNEL_TRACE", "0") == "1"
    tmpdir = os.environ.get("BASS_KERNEL_TRACE_DIR") or None
    if tmpdir:
        os.makedirs(tmpdir, exist_ok=True)
    results = run_bass_kernel_spmd(nc, in_maps, core_ids=list(range(NCORES)),
                                   trace=trace, tmpdir=tmpdir)
    LAST_RESULTS = results

    out = np.empty((B, CO, H, W), np.float32)
    outv = out.reshape(B, CO, NCH, CHUNK)
    for core in range(NCORES):
        yc = results.results[core]["y"]
        for ul in range(UPC):
            un = core * UPC + ul
            b, c = un // NCH, un % NCH
            outv[b, :, c, :] = np.asarray(yc[ul], np.float32)
    return out


if __name__ == "__main__":
    rng = np.random.default_rng(0)
    inputs = {
        "lidar_features": rng.standard_normal((B, CL, H, W), np.float32),
        "image_features": rng.standard_normal((B, CI, H, W), np.float32),
        "lidar_w": rng.standard_normal((CO, CL), np.float32) * np.sqrt(2.0 / CO),
        "lidar_gamma": np.ones(CO, np.float32),
        "lidar_beta": np.zeros(CO, np.float32),
        "lidar_mean": rng.standard_normal(CO).astype(np.float32) * 0.1,
        "lidar_var": rng.uniform(0.5, 1.5, CO).astype(np.float32),
        "image_w": rng.standard_normal((CO, CI), np.float32) * np.sqrt(2.0 / CO),
        "image_gamma": np.ones(CO, np.float32),
        "image_beta": np.zeros(CO, np.float32),
        "image_mean": rng.standard_normal(CO).astype(np.float32) * 0.1,
        "image_var": rng.uniform(0.5, 1.5, CO).astype(np.float32),
        "modality_weights": np.ones(2, np.float32),
    }
    out = kernel(**inputs)
    print("kernel out:", out.shape, out.dtype, float(np.abs(out).mean()))


# revision 10
# speedup vs baseline: 1.6872x; 1.3752x over previous
"""CrossAttentionFusion kernel for Trainium2 (8 NeuronCores, Bass/Tile).

Computation (matches the reference nn.Module):
  image_proj = relu(BN(1x1conv(image_features, image_w)))   # (B,128,H,W)
  lidar_proj = relu(BN(1x1conv(lidar_features, lidar_w)))   # (B,128,H,W)
  per (batch, 2048-pixel chunk): q = image_proj, k = v = lidar_proj
  attn_out = softmax(q k^T / sqrt(128)) @ k
  out = w0 * image_proj + w1 * attn_out,  w = softmax(modality_weights)

Sharding: the 16 independent (batch, chunk) attention problems are
distributed 2-per-core across 8 cores; each core also computes the
projections for its own pixels.  Host gathers the 8 outputs.

Per-core kernel layout notes (v2, bf16):
  - All matmul operands are bf16 (fp32 PSUM accumulate): halves the
    LDWEIGHTS cost, halves DMA, and avoids the fp32r power throttle.
  - w0 is folded into the image BN affine, w1 into the lidar BN affine;
    the exp scale compensates with 1/(w0*w1*sqrt(C)).  The final combine
    is then res = po/denom + qT.
  - Scores are k-major: sT[kpix, q] via matmul with both operands
    channel-major.  exp() on ACT writes bf16 ET.
  - The softmax denominator accumulates S += ET_i on the (otherwise
    idle) Pool engine in f32r, then ones^T @ S broadcast-sums across
    partitions on the PE.
  - AV uses transposed-K bf16 tiles: po[c, q] += Kpix_j^T @ ET_j.
  - Output is written bf16 and cast to fp32 on the host.
"""

import math
import os
import sys
from contextlib import ExitStack

import numpy as np

sys.path.insert(0, "/opt/trn_rl_repo")

import concourse.bass as bass  # noqa: E402
import concourse.tile as tile  # noqa: E402
from concourse import bacc, mybir  # noqa: E402
from concourse.bass import ds, ts  # noqa: E402
from concourse.bass_utils import run_bass_kernel_spmd  # noqa: E402

F32 = mybir.dt.float32
F32R = mybir.dt.float32r
BF16 = mybir.dt.bfloat16

B, CL, CI, CO = 2, 256, 512, 128
H = W = 128
P = H * W                    # 16384 pixels per batch
CHUNK = 2048                 # attention chunk (pixels)
NCH = P // CHUNK             # 8 chunks per batch
NCORES = 8
UPC = (B * NCH) // NCORES    # units (b,chunk) per core = 2
EPS = 1e-5
QB = 1024                    # q-block width (2 matmul halves of 512)
NQB = CHUNK // QB            # 2
KSL = CHUNK // 128           # 16 k-pixel slices per chunk
NCI_IMG = CI // 128          # 4 contraction slices for image proj
NCI_LID = CL // 128          # 2 for lidar proj

_PROGRAM = None              # compiled Bass program, built once per process
LAST_RESULTS = None          # BassKernelResults of the last kernel() call


def _build_program():
    nc = bacc.Bacc("TRN2", target_bir_lowering=False, debug=False,
                   num_devices=NCORES)

    # Per-core DRAM inputs (pre-sharded on host, bf16).
    ximg = nc.dram_tensor("ximg", [UPC, NCI_IMG, 128, CHUNK], BF16,
                          kind="ExternalInput").ap()
    xlid = nc.dram_tensor("xlid", [UPC, NCI_LID, 128, CHUNK], BF16,
                          kind="ExternalInput").ap()
    wimg = nc.dram_tensor("wimg", [NCI_IMG, 128, CO], BF16,
                          kind="ExternalInput").ap()
    wlid = nc.dram_tensor("wlid", [NCI_LID, 128, CO], BF16,
                          kind="ExternalInput").ap()
    img_scale = nc.dram_tensor("img_scale", [CO, 1], F32, kind="ExternalInput").ap()
    img_bias = nc.dram_tensor("img_bias", [CO, 1], F32, kind="ExternalInput").ap()
    lid_scale = nc.dram_tensor("lid_scale", [CO, 1], F32, kind="ExternalInput").ap()
    lid_bias = nc.dram_tensor("lid_bias", [CO, 1], F32, kind="ExternalInput").ap()
    escale = nc.dram_tensor("escale", [128, 1], F32, kind="ExternalInput").ap()
    ident = nc.dram_tensor("ident", [128, 128], BF16, kind="ExternalInput").ap()
    ones_m = nc.dram_tensor("ones_m", [128, 128], BF16, kind="ExternalInput").ap()
    y = nc.dram_tensor("y", [UPC, CO, CHUNK], BF16, kind="ExternalOutput").ap()

    with tile.TileContext(nc) as tc, ExitStack() as ctx:
        const = ctx.enter_context(tc.tile_pool(name="const", bufs=1))
        xi_pool = ctx.enter_context(tc.tile_pool(name="xi", bufs=UPC * NCI_IMG))
        xl_pool = ctx.enter_context(tc.tile_pool(name="xl", bufs=UPC * NCI_LID))
        proj_pool = ctx.enter_context(tc.tile_pool(name="proj", bufs=4))
        kp_pool = ctx.enter_context(tc.tile_pool(name="kp", bufs=4))
        et_pool = ctx.enter_context(tc.tile_pool(name="et", bufs=6))
        s_pool = ctx.enter_context(tc.tile_pool(name="s", bufs=4))
        misc_pool = ctx.enter_context(tc.tile_pool(name="misc", bufs=2))
        res_pool = ctx.enter_context(tc.tile_pool(name="res", bufs=2))
        # PSUM: mm 2x[128,1024]f32 (4 banks) + av 2x[128,1024]f32 (4 banks).
        # Transposes (bf16) and the denominator matmul borrow mm slots.
        mm_psum = ctx.enter_context(tc.tile_pool(name="mmps", bufs=2, space="PSUM"))
        av_psum = ctx.enter_context(tc.tile_pool(name="avps", bufs=2, space="PSUM"))

        # constants
        wimg_t = const.tile([128, NCI_IMG * CO], BF16)
        for ci in range(NCI_IMG):
            nc.sync.dma_start(wimg_t[:, ts(ci, CO)], wimg[ci])
        wlid_t = const.tile([128, NCI_LID * CO], BF16)
        for ci in range(NCI_LID):
            nc.sync.dma_start(wlid_t[:, ts(ci, CO)], wlid[ci])
        img_s = const.tile([128, 1], F32)
        nc.sync.dma_start(img_s[:], img_scale)
        img_b = const.tile([128, 1], F32)
        nc.sync.dma_start(img_b[:], img_bias)
        lid_s = const.tile([128, 1], F32)
        nc.sync.dma_start(lid_s[:], lid_scale)
        lid_b = const.tile([128, 1], F32)
        nc.sync.dma_start(lid_b[:], lid_bias)
        esc = const.tile([128, 1], F32)
        nc.sync.dma_start(esc[:], escale)
        ident_t = const.tile([128, 128], BF16)
        nc.sync.dma_start(ident_t[:], ident)
        ones_t = const.tile([128, 128], BF16)
        nc.sync.dma_start(ones_t[:], ones_m)

        # ---- prefetch ALL unit inputs up front so the in-order DMA queue
        # never serializes later units' loads behind the y output DMA; the
        # first q-block's halves go first so proj can start ASAP ----
        xi_u, xl_u = [], []
        for u in range(UPC):
            xi = [xi_pool.tile([128, CHUNK], BF16, name=f"xi_{u}_{ci}", tag="xi")
                  for ci in range(NCI_IMG)]
            xl = [xl_pool.tile([128, CHUNK], BF16, name=f"xl_{u}_{ci}", tag="xl")
                  for ci in range(NCI_LID)]
            for hh in range(2):
                for ci in range(NCI_IMG):
                    nc.sync.dma_start(xi[ci][:, ts(hh, QB)],
                                      ximg[u, ci, :, ts(hh, QB)])
                for ci in range(NCI_LID):
                    nc.sync.dma_start(xl[ci][:, ts(hh, QB)],
                                      xlid[u, ci, :, ts(hh, QB)])
            xi_u.append(xi)
            xl_u.append(xl)

        for u in range(UPC):
            xi, xl = xi_u[u], xl_u[u]

            # ---- projections (channel-major), QB-wide PSUM, halves of 512 ----
            qT = proj_pool.tile([128, CHUNK], BF16, name=f"qT_{u}", tag="qT")
            kT = proj_pool.tile([128, CHUNK], BF16, name=f"kT_{u}", tag="kT")
            for qb in range(NQB):
                ps = mm_psum.tile([128, QB], F32, name=f"psi_{u}_{qb}", tag="mm")
                for h in range(QB // 512):
                    for ci in range(NCI_IMG):
                        nc.tensor.matmul(ps[:, ts(h, 512)], wimg_t[:, ts(ci, CO)],
                                         xi[ci][:, ds(qb * QB + h * 512, 512)],
                                         start=(ci == 0), stop=(ci == NCI_IMG - 1))
                nc.scalar.activation(qT[:, ts(qb, QB)], ps[:],
                                     mybir.ActivationFunctionType.Relu,
                                     bias=img_b[:], scale=img_s[:])
                ps2 = mm_psum.tile([128, QB], F32, name=f"psl_{u}_{qb}", tag="mm")
                for h in range(QB // 512):
                    for ci in range(NCI_LID):
                        nc.tensor.matmul(ps2[:, ts(h, 512)], wlid_t[:, ts(ci, CO)],
                                         xl[ci][:, ds(qb * QB + h * 512, 512)],
                                         start=(ci == 0), stop=(ci == NCI_LID - 1))
                nc.scalar.activation(kT[:, ts(qb, QB)], ps2[:],
                                     mybir.ActivationFunctionType.Relu,
                                     bias=lid_b[:], scale=lid_s[:])

            # ---- transpose K to pixel-major bf16 tiles (8 transposes per
            # one-bank bf16 PSUM tile borrowed from the mm pool, one wide
            # DVE copy) ----
            kpw = []
            for g in range(KSL // 8):
                pt = mm_psum.tile([128, 8 * 128], BF16, name=f"pt_{u}_{g}",
                                  tag="mm")
                for k in range(8):
                    nc.tensor.transpose(pt[:, ts(k, 128)],
                                        kT[:, ts(g * 8 + k, 128)], ident_t[:])
                kpt = kp_pool.tile([128, 8 * 128], BF16,
                                   name=f"kp_{u}_{g}", tag="kp")
                nc.vector.tensor_copy(kpt[:], pt[:])
                kpw.append(kpt)

            # ---- attention, one q-block at a time ----
            res_u = res_pool.tile([128, CHUNK], BF16, name=f"res_{u}", tag="res")
            LOOKAHEAD = 3  # AV matmuls lag scores so the in-order PE queue
            #                never stalls waiting on ACT-engine exp
            for qb in range(NQB):
                po = av_psum.tile([128, QB], F32, name=f"po_{u}_{qb}", tag="av")
                # softmax denominator: two bf16 partial sums (even/odd k-
                # slices), both on DVE where all-bf16 SBUF ops hit the 2x/4x
                # perf modes; recombined in fp32 by the ones^T matmul
                SA = s_pool.tile([128, QB], BF16, name=f"SA_{u}_{qb}", tag="S")
                SB = s_pool.tile([128, QB], BF16, name=f"SB_{u}_{qb}", tag="S")
                ets = [None] * KSL
                for i in range(KSL + LOOKAHEAD):
                    if i < KSL:
                        ps = mm_psum.tile([128, QB], F32,
                                          name=f"pss_{u}_{qb}_{i}", tag="mm")
                        for h in range(QB // 512):
                            nc.tensor.matmul(ps[:, ts(h, 512)], kT[:, ts(i, 128)],
                                             qT[:, ds(qb * QB + h * 512, 512)],
                                             start=True, stop=True)
                        et = et_pool.tile([128, QB], BF16,
                                          name=f"et_{u}_{qb}_{i}", tag="et")
                        nc.scalar.activation(et[:], ps[:],
                                             mybir.ActivationFunctionType.Exp,
                                             scale=esc[:])
                        ets[i] = et
                        S = SA if i % 2 == 0 else SB
                        if i < 2:
                            nc.vector.tensor_copy(S[:], et[:])
                        else:
                            nc.vector.tensor_add(S[:], S[:], et[:])
                    j = i - LOOKAHEAD
                    if j >= 0:
                        kslice = kpw[j // 8][:, ts(j % 8, 128)]
                        for h in range(QB // 512):
                            nc.tensor.matmul(po[:, ts(h, 512)], kslice,
                                             ets[j][:, ts(h, 512)],
                                             start=(j == 0), stop=(j == KSL - 1))
                pl = mm_psum.tile([128, QB], F32, name=f"pl_{u}_{qb}", tag="mm")
                for h in range(QB // 512):
                    nc.tensor.matmul(pl[:, ts(h, 512)], ones_t[:],
                                     SA[:, ts(h, 512)], start=True, stop=False)
                    nc.tensor.matmul(pl[:, ts(h, 512)], ones_t[:],
                                     SB[:, ts(h, 512)], start=False, stop=True)
                linv = misc_pool.tile([128, QB], F32, name=f"linv_{u}_{qb}",
                                      tag="linv")
                nc.vector.reciprocal_approx_fast(linv[:], pl[:])
                tmp = misc_pool.tile([128, QB], BF16, name=f"tmp_{u}_{qb}",
                                     tag="tmp")
                nc.vector.tensor_mul(tmp[:], po[:], linv[:])
                # res = attn_out + w0*image_proj  (qT has w0, po/S has w1);
                # all-bf16, once per q-block: cheap enough for the Pool engine
                nc.gpsimd.tensor_add(res_u[:, ts(qb, QB)], tmp[:],
                                     qT[:, ts(qb, QB)])
            nc.sync.dma_start(y[u], res_u[:])

    nc.compile()
    return nc


def _to_bf16(a):
    """Round-to-nearest-even fp32 -> bf16, returned as ml_dtypes.bfloat16."""
    import ml_dtypes
    return np.asarray(a, np.float32).astype(ml_dtypes.bfloat16)


def _shard_inputs(inputs):
    """Build the 8 per-core input maps from the full input dict."""
    import ml_dtypes
    mw = np.asarray(inputs["modality_weights"], np.float64)
    e = np.exp(mw - mw.max())
    w = (e / e.sum()).astype(np.float64)
    w0, w1 = float(w[0]), float(w[1])

    def bn_fold(gamma, beta, mean, var, mul):
        g = np.asarray(gamma, np.float64)
        b = np.asarray(beta, np.float64)
        m = np.asarray(mean, np.float64)
        v = np.asarray(var, np.float64)
        scale = g / np.sqrt(v + EPS) * mul
        bias = (b - m * g / np.sqrt(v + EPS)) * mul
        return (scale.astype(np.float32).reshape(CO, 1),
                bias.astype(np.float32).reshape(CO, 1))

    i_s, i_b = bn_fold(inputs["image_gamma"], inputs["image_beta"],
                       inputs["image_mean"], inputs["image_var"], w0)
    l_s, l_b = bn_fold(inputs["lidar_gamma"], inputs["lidar_beta"],
                       inputs["lidar_mean"], inputs["lidar_var"], w1)

    # weight slices, pre-transposed for lhsT ([cin_slice, cout]), bf16
    wi = _to_bf16(np.ascontiguousarray(
        np.asarray(inputs["image_w"], np.float32).T.reshape(NCI_IMG, 128, CO)))
    wl = _to_bf16(np.ascontiguousarray(
        np.asarray(inputs["lidar_w"], np.float32).T.reshape(NCI_LID, 128, CO)))

    esc = np.full((128, 1), 1.0 / (w0 * w1 * math.sqrt(CO)), np.float32)
    ident = np.eye(128, dtype=ml_dtypes.bfloat16)
    ones_m = np.ones((128, 128), ml_dtypes.bfloat16)

    # full features reshaped to (B, nchunks, C, 2048), cast once to bf16
    img = _to_bf16(np.asarray(inputs["image_features"], np.float32)
                   ).reshape(B, CI, NCH, CHUNK)
    lid = _to_bf16(np.asarray(inputs["lidar_features"], np.float32)
                   ).reshape(B, CL, NCH, CHUNK)

    in_maps = []
    for core in range(NCORES):
        ximg = np.empty((UPC, NCI_IMG, 128, CHUNK), ml_dtypes.bfloat16)
        xlid = np.empty((UPC, NCI_LID, 128, CHUNK), ml_dtypes.bfloat16)
        for ul in range(UPC):
            un = core * UPC + ul
            b, c = un // NCH, un % NCH
            ximg[ul] = img[b, :, c, :].reshape(NCI_IMG, 128, CHUNK)
            xlid[ul] = lid[b, :, c, :].reshape(NCI_LID, 128, CHUNK)
        in_maps.append({
            "ximg": ximg, "xlid": xlid, "wimg": wi, "wlid": wl,
            "img_scale": i_s, "img_bias": i_b,
            "lid_scale": l_s, "lid_bias": l_b,
            "escale": esc, "ident": ident, "ones_m": ones_m,
        })
    return in_maps


def kernel(**inputs) -> np.ndarray:
    global _PROGRAM, LAST_RESULTS
    if _PROGRAM is None:
        _PROGRAM = _build_program()
    nc = _PROGRAM

    in_maps = _shard_inputs(inputs)
    trace = os.environ.get("BASS_KERNEL_TRACE", "0") == "1"
    tmpdir = os.environ.get("BASS_KERNEL_TRACE_DIR") or None
    if tmpdir:
        os.makedirs(tmpdir, exist_ok=True)
    results = run_bass_kernel_spmd(nc, in_maps, core_ids=list(range(NCORES)),
                                   trace=trace, tmpdir=tmpdir)
    LAST_RESULTS = results

    out = np.empty((B, CO, H, W), np.float32)
    outv = out.reshape(B, CO, NCH, CHUNK)
    for core in range(NCORES):
        yc = results.results[core]["y"]
        for ul in range(UPC):
            un = core * UPC + ul
            b, c = un // NCH, un % NCH
            outv[b, :, c, :] = np.asarray(yc[ul], np.float32)
    return out


if __name__ == "__main__":
    rng = np.random.default_rng(0)
    inputs = {
        "lidar_features": rng.standard_normal((B, CL, H, W), np.float32),
        "image_features": rng.standard_normal((B, CI, H, W), np.float32),
        "lidar_w": rng.standard_normal((CO, CL), np.float32) * np.sqrt(2.0 / CO),
        "lidar_gamma": np.ones(CO, np.float32),
        "lidar_beta": np.zeros(CO, np.float32),
        "lidar_mean": rng.standard_normal(CO).astype(np.float32) * 0.1,
        "lidar_var": rng.uniform(0.5, 1.5, CO).astype(np.float32),
        "image_w": rng.standard_normal((CO, CI), np.float32) * np.sqrt(2.0 / CO),
        "image_gamma": np.ones(CO, np.float32),
        "image_beta": np.zeros(CO, np.float32),
        "image_mean": rng.standard_normal(CO).astype(np.float32) * 0.1,
        "image_var": rng.uniform(0.5, 1.5, CO).astype(np.float32),
        "modality_weights": np.ones(2, np.float32),
    }
    out = kernel(**inputs)
    print("kernel out:", out.shape, out.dtype, float(np.abs(out).mean()))
